# revision 21
# baseline (speedup 1.0000x reference)
"""EyeLoss Trainium2 kernel (nn_EyeLoss_83691732730572).

Key observation: the reference only ever consumes channels 96 and 97 of the
98-channel heatmaps (eyes = landmarks[:, 96/97], MSE over [:, 96/97]), so of
the 2 x 128 x 98 x 64 x 64 f32 input only 8 MB is live. The kernel shards
batches across 8 NeuronCores (16 per core) and ships only the two live
channels. Per core, each (batch, channel, tensor) 64x64 map is split into two
2048-element half-maps, one per partition (q = 2m + h, so each tensor's load
is a single constant-stride 2D DMA), and the device computes:

  - argmax per half-map: DVE Max8 + MaxIndex (value + first-match index)
  - one indirect-DMA gather of the 129-element window centred on the argmax
    (one offset per partition, contiguous row) -> all four +-1 / +-64
    neighbors needed for the subpixel refinement
  - the MSE sum of (src - tgt)^2 via an aligned [128 x 1024] reshape load,
    one DVE subtract, and a DVE fused square-with-accumulate

Each core returns one packed [128, 140] f32 tile (8 meta cols + the gathered
window); the host finishes the ~100 flops of subpixel decode per map and the
scalar mean reduction, in the same f32 arithmetic as the reference, so the
eye outputs are bit-exact.
"""
import numpy as np

import concourse.bass as bass
import concourse.mybir as mybir
from concourse.bass_utils import run_bass_kernel_spmd

F32 = mybir.dt.float32
U32 = mybir.dt.uint32
I32 = mybir.dt.int32

B, L, H, W = 128, 98, 64, 64
HW = H * W                      # 4096
N_CORES = 8
BPC = B // N_CORES              # 16 batches per core
NMAP = BPC * 2                  # 32 maps per tensor per core (b-major, l minor)
HALF = HW // 2                  # 2048 elems per half-map
QT = HALF // 2                  # 1024-elem rows for the MSE reshape
HALO = W                        # one image row of halo on each side
ROW = HALF                      # 2048 cols per partition (no halos needed)
DATA1 = NMAP * HW               # one tensor's flat length (131072)
CLEN = HALO + 2 * DATA1 + HALO  # [64 z | src flat | tgt flat | 64 z]
GW = 2 * W + 1                  # gathered window width (129)
PKW = 8 + GW + 3                # packed output width (140)

# Optional tracing knobs (used by test.py; harness leaves these alone).
TRACE = False
LAST_RESULTS = None

_CACHED_NC = None


def _build_nc():
    nc = bass.Bass()
    data = nc.dram_tensor("data", [CLEN, 1], F32, kind="ExternalInput")
    base1 = nc.dram_tensor("base1", [128, 1], F32, kind="ExternalInput")
    outp = nc.dram_tensor("outp", [128, PKW], F32, kind="ExternalOutput")

    from contextlib import ExitStack

    with ExitStack() as ctx:
        T = ctx.enter_context(nc.sbuf_tensor("T", [128, ROW], F32))
        S2 = ctx.enter_context(nc.sbuf_tensor("S2", [128, QT], F32))
        T2 = ctx.enter_context(nc.sbuf_tensor("T2", [128, QT], F32))
        D2 = ctx.enter_context(nc.sbuf_tensor("D2", [128, QT], F32))
        SQ = ctx.enter_context(nc.sbuf_tensor("SQ", [128, QT], F32))
        MAX8 = ctx.enter_context(nc.sbuf_tensor("MAX8", [128, 8], F32))
        IDX8 = ctx.enter_context(nc.sbuf_tensor("IDX8", [128, 8], U32))
        BASE = ctx.enter_context(nc.sbuf_tensor("BASE", [128, 1], F32))
        IDXF = ctx.enter_context(nc.sbuf_tensor("IDXF", [128, 1], F32))
        OFF = ctx.enter_context(nc.sbuf_tensor("OFF", [128, 1], I32))
        PK = ctx.enter_context(nc.sbuf_tensor("PK", [128, PKW], F32))
        ZB = ctx.enter_context(nc.sbuf_tensor("ZB", [128, 1], F32))
        sem_T = ctx.enter_context(nc.semaphore("sem_T"))
        sem_B = ctx.enter_context(nc.semaphore("sem_B"))
        sem_S = ctx.enter_context(nc.semaphore("sem_S"))
        sem_max8 = ctx.enter_context(nc.semaphore("sem_max8"))
        sem_idx = ctx.enter_context(nc.semaphore("sem_idx"))
        sem_idxf = ctx.enter_context(nc.semaphore("sem_idxf"))
        sem_off = ctx.enter_context(nc.semaphore("sem_off"))
        sem_g = ctx.enter_context(nc.semaphore("sem_g"))
        sem_d = ctx.enter_context(nc.semaphore("sem_d"))
        sem_act = ctx.enter_context(nc.semaphore("sem_act"))
        dma_out = ctx.enter_context(nc.semaphore("dma_out"))
        block = ctx.enter_context(nc.Block())

        # T partition p = t*64 + q with q = 2m + h = 4b + 2l + h: the (m, h)
        # half-map windows start at q*2048 in DRAM, so each tensor's load is a
        # single constant-stride 2D DMA (windows overlap by the halos).
        # T cols: [0,64) left halo | [64, 2112) data | [2112, 2176) right halo
        # data DRAM: [64 z | src maps | 64 z | 64 z | tgt maps | 64 z]
        @block.sync
        def _(sync):
            sync.dma_start(
                T[:, :],
                bass.AP(data, HALO, [[HALF, 128], [1, ROW]]),
            ).then_inc(sem_T, 16)
            sync.dma_start(BASE[:, :], base1[:, :]).then_inc(sem_B, 16)
            sync.wait_ge(sem_d, 1)              # DVE done (mx/idx written)
            sync.wait_ge(sem_act, 1)            # ms written
            sync.wait_ge(sem_g, 16)             # gather window written
            sync.dma_start(outp[:, :], PK[:, :]).then_inc(dma_out, 16)
            # no explicit receipt wait: the Block-exit drain flushes the
            # HWDGE ring before the NEFF-final all-engine barrier

        @block.vector
        def _(vector):
            vector.memset(PK[:, :], 0.0)
            vector.memset(ZB[:, :], 0.0)
            vector.wait_ge(sem_T, 16)           # T fully loaded
            vector.max(MAX8[:, :], T[:, :]).then_inc(sem_max8, 1)
            vector.wait_ge(sem_max8, 1)
            vector.tensor_copy(PK[:, 0:1], MAX8[:, 0:1])
            vector.max_index(
                IDX8[:, :], MAX8[:, :], T[:, :]
            ).then_inc(sem_idx, 1)
            vector.wait_ge(sem_idx, 1)
            vector.tensor_copy(PK[:, 1:2], IDX8[:, 0:1])
            vector.wait_ge(sem_S, 32)           # S2/T2 loaded
            vector.tensor_tensor(
                D2[:, :], S2[:, :], T2[:, :], op=mybir.AluOpType.subtract
            ).then_inc(sem_d, 1)
            vector.wait_ge(sem_d, 1)
            vector.scalar_tensor_tensor(
                SQ[:, :], D2[:, :], 0.0, D2[:, :],
                op0=mybir.AluOpType.add, op1=mybir.AluOpType.mult,
                accum_out=PK[:, 2:3],
            ).then_inc(sem_act, 1)

        @block.gpsimd
        def _(gpsimd):
            gpsimd.wait_ge(sem_B, 16)
            gpsimd.wait_ge(sem_idx, 1)
            # window start: OFF[p] = BASE[p] + argmax_index[p]
            # (f32 math, values < 2^24 so exact; int32 on output write)
            gpsimd.tensor_copy(IDXF[:, :], IDX8[:, 0:1]).then_inc(sem_idxf, 1)
            gpsimd.wait_ge(sem_idxf, 1)
            gpsimd.tensor_scalar(
                OFF[:, :], BASE[:, :], IDXF[:, 0:1], None,
                op0=mybir.AluOpType.add,
            ).then_inc(sem_off, 1)
            gpsimd.wait_ge(sem_off, 1)
            # one gather: PK[p, 8:137] = data[OFF[p] : OFF[p] + 129]
            gpsimd.indirect_dma_start(
                out=PK[:, 8 : 8 + GW],
                out_offset=None,
                in_=data[:, :],
                in_offset=bass.IndirectOffsetOnAxis(ap=OFF[:, 0:1], axis=0),
            ).then_inc(sem_g, 16)

        @block.scalar
        def _(scalar):
            # MSE reshape loads gated on the T load so their SDMA packets
            # don't round-robin-starve the critical T load.
            # S2/T2 = the tensor's whole data region viewed as [128, 1024].
            scalar.wait_ge(sem_T, 16)
            for t, dst in ((0, S2), (1, T2)):
                scalar.dma_start(
                    dst[:, :],
                    bass.AP(data, HALO + t * DATA1, [[QT, 128], [1, QT]]),
                ).then_inc(sem_S, 16)           # -> 32

    return nc


def _base1():
    # gather window start: buffer pos (HALO + p*2048 + idx) - 64 = p*2048 + idx
    p = np.arange(128)
    return (p * HALF).astype(np.float32).reshape(128, 1)


_BASE1 = _base1()


def kernel(source_heatmap, target_heatmap):
    global _CACHED_NC, LAST_RESULTS
    src = np.asarray(source_heatmap, np.float32)
    tgt = np.asarray(target_heatmap, np.float32)

    # per-core inputs: batches [c*16, (c+1)*16), channels 96..97, flattened
    in_maps = []
    for c in range(N_CORES):
        buf = np.zeros(CLEN, np.float32)
        s = np.ascontiguousarray(src[c * BPC : (c + 1) * BPC, 96:98]).reshape(-1)
        t = np.ascontiguousarray(tgt[c * BPC : (c + 1) * BPC, 96:98]).reshape(-1)
        buf[HALO : HALO + DATA1] = s
        buf[HALO + DATA1 : HALO + 2 * DATA1] = t
        in_maps.append({"data": buf.reshape(CLEN, 1), "base1": _BASE1})

    if _CACHED_NC is None:
        _CACHED_NC = _build_nc()
    res = run_bass_kernel_spmd(
        _CACHED_NC, in_maps, list(range(N_CORES)), trace=TRACE
    )
    LAST_RESULTS = res

    # ---- host decode of the packed [128, 140] per-core outputs ----
    # partition p = t*64 + 4*b_local + 2*l + h  (l: 0 -> ch96, 1 -> ch97)
    # cols: 0=mx 1=idx 2=ms 3..7 pad, 8+c = flat[idx - 64 + c] for c in [0,129)
    pk = np.stack([res.results[c]["outp"] for c in range(N_CORES)])  # [8,128,PKW]

    pk32 = pk.astype(np.float32)
    dx32 = pk32[:, :, 8 + W + 1] - pk32[:, :, 8 + W - 1]   # f32, same rounding as ref
    dy32 = pk32[:, :, 8 + 2 * W] - pk32[:, :, 8 + 0]

    # [core, t, m, h] with m = 2*b_local + l
    v = pk.astype(np.float64)[:, :, 0:2].reshape(N_CORES, 2, 32, 2, 2)
    mx = v[..., 0]
    idx = v[..., 1]
    dx = dx32.astype(np.float64).reshape(N_CORES, 2, 32, 2)
    dy = dy32.astype(np.float64).reshape(N_CORES, 2, 32, 2)

    hwin = (mx[:, :, :, 1] > mx[:, :, :, 0]).astype(np.int64)  # [core, t, m]
    sel = np.take_along_axis
    idx_w = sel(idx, hwin[:, :, :, None], axis=3)[:, :, :, 0]
    dx_w = sel(dx, hwin[:, :, :, None], axis=3)[:, :, :, 0]
    dy_w = sel(dy, hwin[:, :, :, None], axis=3)[:, :, :, 0]

    flat = hwin * HALF + idx_w.astype(np.int64)    # [core, t, m] in [0, 4096)
    px = (flat % W).astype(np.float32)
    py = (flat // W).astype(np.float32)
    inside = (px > 0) & (px < W - 1) & (py > 0) & (py < H - 1)
    off_x = np.where(inside, np.sign(dx_w).astype(np.float32) * 0.25, 0.0).astype(np.float32)
    off_y = np.where(inside, np.sign(dy_w).astype(np.float32) * 0.25, 0.0).astype(np.float32)
    lx = (px + 0.5 + off_x) * 4.0                  # landmark x
    ly = (py + 0.5 + off_y) * 4.0

    # eyes[b] = [x96, y96, x97, y97];  m = 2*b_local + l
    lx = lx.reshape(N_CORES, 2, BPC, 2)            # [core, t, b_local, l]
    ly = ly.reshape(N_CORES, 2, BPC, 2)
    eyes = np.empty((2, B, 4), np.float32)
    for t in range(2):
        exy = np.stack(
            [lx[:, t, :, 0], ly[:, t, :, 0], lx[:, t, :, 1], ly[:, t, :, 1]],
            axis=-1,
        )                                          # [core, b_local, 4]
        eyes[t] = exy.reshape(B, 4)

    # MSE: col 2, partition r = 8b + 4l + 2h + g  ->  l = (r >> 2) & 1
    ms = pk.astype(np.float64)[:, :, 2].reshape(N_CORES, BPC, 2, 4)
    left = ms[:, :, 0, :].sum() / (B * HW)
    right = ms[:, :, 1, :].sum() / (B * HW)
    eye_loss = np.float32(left + right)

    return eye_loss, eyes[0], eyes[1]


# revision 22
# speedup vs baseline: 1.0443x; 1.0443x over previous
"""EyeLoss Trainium2 kernel (nn_EyeLoss_83691732730572).

Key observation: the reference only ever consumes channels 96 and 97 of the
98-channel heatmaps (eyes = landmarks[:, 96/97], MSE over [:, 96/97]), so of
the 2 x 128 x 98 x 64 x 64 f32 input only 8 MB is live. The kernel shards
batches across 8 NeuronCores (16 per core) and ships only the two live
channels. Per core, each (batch, channel, tensor) 64x64 map is split into two
2048-element half-maps, one per partition (q = 2m + h, so each tensor's load
is a single constant-stride 2D DMA), and the device computes:

  - argmax per half-map: DVE Max8 + MaxIndex (value + first-match index)
  - one indirect-DMA gather of the 129-element window centred on the argmax
    (one offset per partition, contiguous row) -> all four +-1 / +-64
    neighbors needed for the subpixel refinement
  - the MSE sum of (src - tgt)^2 via an aligned [128 x 1024] reshape load,
    one DVE subtract, and a DVE fused square-with-accumulate

Each core returns one packed [128, 140] f32 tile (8 meta cols + the gathered
window); the host finishes the ~100 flops of subpixel decode per map and the
scalar mean reduction, in the same f32 arithmetic as the reference, so the
eye outputs are bit-exact.
"""
import numpy as np

import concourse.bass as bass
import concourse.mybir as mybir
from concourse.bass_utils import run_bass_kernel_spmd

F32 = mybir.dt.float32
U32 = mybir.dt.uint32
I32 = mybir.dt.int32

B, L, H, W = 128, 98, 64, 64
HW = H * W                      # 4096
N_CORES = 8
BPC = B // N_CORES              # 16 batches per core
NMAP = BPC * 2                  # 32 maps per tensor per core (b-major, l minor)
HALF = HW // 2                  # 2048 elems per half-map
QT = HALF // 2                  # 1024-elem rows for the MSE reshape
HALO = W                        # one image row of halo on each side
ROW = HALF                      # 2048 cols per partition (no halos needed)
DATA1 = NMAP * HW               # one tensor's flat length (131072)
CLEN = HALO + 2 * DATA1 + HALO  # [64 z | src flat | tgt flat | 64 z]
GW = 2 * W + 1                  # gathered window width (129)
PKW = 8 + GW + 3                # packed output width (140)

# Optional tracing knobs (used by test.py; harness leaves these alone).
TRACE = False
LAST_RESULTS = None

_CACHED_NC = None


def _build_nc():
    nc = bass.Bass()
    data = nc.dram_tensor("data", [CLEN, 1], F32, kind="ExternalInput")
    base1 = nc.dram_tensor("base1", [128, 1], F32, kind="ExternalInput")
    outp = nc.dram_tensor("outp", [128, PKW], F32, kind="ExternalOutput")

    from contextlib import ExitStack

    with ExitStack() as ctx:
        T = ctx.enter_context(nc.sbuf_tensor("T", [128, ROW], F32))
        S2 = ctx.enter_context(nc.sbuf_tensor("S2", [128, QT], F32))
        T2 = ctx.enter_context(nc.sbuf_tensor("T2", [128, QT], F32))
        D2 = ctx.enter_context(nc.sbuf_tensor("D2", [128, QT], F32))
        SQ = ctx.enter_context(nc.sbuf_tensor("SQ", [128, QT], F32))
        MAX8 = ctx.enter_context(nc.sbuf_tensor("MAX8", [128, 8], F32))
        IDX8 = ctx.enter_context(nc.sbuf_tensor("IDX8", [128, 8], U32))
        BASE = ctx.enter_context(nc.sbuf_tensor("BASE", [128, 1], F32))
        IDXF = ctx.enter_context(nc.sbuf_tensor("IDXF", [128, 1], F32))
        OFF = ctx.enter_context(nc.sbuf_tensor("OFF", [128, 1], I32))
        PK = ctx.enter_context(nc.sbuf_tensor("PK", [128, PKW], F32))
        ZB = ctx.enter_context(nc.sbuf_tensor("ZB", [128, 1], F32))
        sem_T = ctx.enter_context(nc.semaphore("sem_T"))
        sem_B = ctx.enter_context(nc.semaphore("sem_B"))
        sem_S = ctx.enter_context(nc.semaphore("sem_S"))
        sem_max8 = ctx.enter_context(nc.semaphore("sem_max8"))
        sem_idx = ctx.enter_context(nc.semaphore("sem_idx"))
        sem_idxf = ctx.enter_context(nc.semaphore("sem_idxf"))
        sem_off = ctx.enter_context(nc.semaphore("sem_off"))
        sem_g = ctx.enter_context(nc.semaphore("sem_g"))
        sem_d = ctx.enter_context(nc.semaphore("sem_d"))
        sem_act = ctx.enter_context(nc.semaphore("sem_act"))
        dma_out = ctx.enter_context(nc.semaphore("dma_out"))
        block = ctx.enter_context(nc.Block())

        # T partition p = t*64 + q with q = 2m + h = 4b + 2l + h: the (m, h)
        # half-map windows start at q*2048 in DRAM, so each tensor's load is a
        # single constant-stride 2D DMA (windows overlap by the halos).
        # T cols: [0,64) left halo | [64, 2112) data | [2112, 2176) right halo
        # data DRAM: [64 z | src maps | 64 z | 64 z | tgt maps | 64 z]
        @block.sync
        def _(sync):
            sync.dma_start(
                T[:, :],
                bass.AP(data, HALO, [[HALF, 128], [1, ROW]]),
            ).then_inc(sem_T, 16)
            sync.dma_start(BASE[:, :], base1[:, :]).then_inc(sem_B, 16)
            sync.wait_ge(sem_d, 1)              # DVE done (mx/idx written)
            sync.wait_ge(sem_act, 1)            # ms written
            sync.wait_ge(sem_g, 16)             # gather window written
            sync.dma_start(outp[:, :], PK[:, :]).then_inc(dma_out, 16)
            # no explicit receipt wait: the Block-exit drain flushes the
            # HWDGE ring before the NEFF-final all-engine barrier

        @block.vector
        def _(vector):
            vector.memset(PK[:, :], 0.0)
            vector.memset(ZB[:, :], 0.0)
            vector.wait_ge(sem_T, 16)           # T fully loaded
            vector.max(MAX8[:, :], T[:, :]).then_inc(sem_max8, 1)
            vector.wait_ge(sem_max8, 1)
            vector.tensor_copy(PK[:, 0:1], MAX8[:, 0:1])
            vector.max_index(
                IDX8[:, :], MAX8[:, :], T[:, :]
            ).then_inc(sem_idx, 1)
            vector.wait_ge(sem_idx, 1)
            vector.tensor_copy(IDXF[:, :], IDX8[:, 0:1]).then_inc(sem_idxf, 1)
            vector.tensor_copy(PK[:, 1:2], IDX8[:, 0:1])
            vector.wait_ge(sem_B, 16)
            vector.wait_ge(sem_idxf, 1)
            # window start: OFF[p] = BASE[p] + argmax_index[p]
            # (f32 math, values < 2^24 so exact; int32 on output write)
            vector.tensor_scalar(
                OFF[:, :], BASE[:, :], IDXF[:, 0:1], None,
                op0=mybir.AluOpType.add,
            ).then_inc(sem_off, 1)
            vector.wait_ge(sem_S, 32)           # S2/T2 loaded
            vector.tensor_tensor(
                D2[:, :], S2[:, :], T2[:, :], op=mybir.AluOpType.subtract
            ).then_inc(sem_d, 1)
            vector.wait_ge(sem_d, 1)
            vector.scalar_tensor_tensor(
                SQ[:, :], D2[:, :], 0.0, D2[:, :],
                op0=mybir.AluOpType.add, op1=mybir.AluOpType.mult,
                accum_out=PK[:, 2:3],
            ).then_inc(sem_act, 1)

        @block.gpsimd
        def _(gpsimd):
            gpsimd.wait_ge(sem_off, 1)
            # one gather: PK[p, 8:137] = data[OFF[p] : OFF[p] + 129]
            gpsimd.indirect_dma_start(
                out=PK[:, 8 : 8 + GW],
                out_offset=None,
                in_=data[:, :],
                in_offset=bass.IndirectOffsetOnAxis(ap=OFF[:, 0:1], axis=0),
            ).then_inc(sem_g, 16)

        @block.scalar
        def _(scalar):
            # MSE reshape loads gated on the T load so their SDMA packets
            # don't round-robin-starve the critical T load.
            # S2/T2 = the tensor's whole data region viewed as [128, 1024].
            scalar.wait_ge(sem_T, 16)
            for t, dst in ((0, S2), (1, T2)):
                scalar.dma_start(
                    dst[:, :],
                    bass.AP(data, HALO + t * DATA1, [[QT, 128], [1, QT]]),
                ).then_inc(sem_S, 16)           # -> 32

    return nc


def _base1():
    # gather window start: buffer pos (HALO + p*2048 + idx) - 64 = p*2048 + idx
    p = np.arange(128)
    return (p * HALF).astype(np.float32).reshape(128, 1)


_BASE1 = _base1()


def kernel(source_heatmap, target_heatmap):
    global _CACHED_NC, LAST_RESULTS
    src = np.asarray(source_heatmap, np.float32)
    tgt = np.asarray(target_heatmap, np.float32)

    # per-core inputs: batches [c*16, (c+1)*16), channels 96..97, flattened
    in_maps = []
    for c in range(N_CORES):
        buf = np.zeros(CLEN, np.float32)
        s = np.ascontiguousarray(src[c * BPC : (c + 1) * BPC, 96:98]).reshape(-1)
        t = np.ascontiguousarray(tgt[c * BPC : (c + 1) * BPC, 96:98]).reshape(-1)
        buf[HALO : HALO + DATA1] = s
        buf[HALO + DATA1 : HALO + 2 * DATA1] = t
        in_maps.append({"data": buf.reshape(CLEN, 1), "base1": _BASE1})

    if _CACHED_NC is None:
        _CACHED_NC = _build_nc()
    res = run_bass_kernel_spmd(
        _CACHED_NC, in_maps, list(range(N_CORES)), trace=TRACE
    )
    LAST_RESULTS = res

    # ---- host decode of the packed [128, 140] per-core outputs ----
    # partition p = t*64 + 4*b_local + 2*l + h  (l: 0 -> ch96, 1 -> ch97)
    # cols: 0=mx 1=idx 2=ms 3..7 pad, 8+c = flat[idx - 64 + c] for c in [0,129)
    pk = np.stack([res.results[c]["outp"] for c in range(N_CORES)])  # [8,128,PKW]

    pk32 = pk.astype(np.float32)
    dx32 = pk32[:, :, 8 + W + 1] - pk32[:, :, 8 + W - 1]   # f32, same rounding as ref
    dy32 = pk32[:, :, 8 + 2 * W] - pk32[:, :, 8 + 0]

    # [core, t, m, h] with m = 2*b_local + l
    v = pk.astype(np.float64)[:, :, 0:2].reshape(N_CORES, 2, 32, 2, 2)
    mx = v[..., 0]
    idx = v[..., 1]
    dx = dx32.astype(np.float64).reshape(N_CORES, 2, 32, 2)
    dy = dy32.astype(np.float64).reshape(N_CORES, 2, 32, 2)

    hwin = (mx[:, :, :, 1] > mx[:, :, :, 0]).astype(np.int64)  # [core, t, m]
    sel = np.take_along_axis
    idx_w = sel(idx, hwin[:, :, :, None], axis=3)[:, :, :, 0]
    dx_w = sel(dx, hwin[:, :, :, None], axis=3)[:, :, :, 0]
    dy_w = sel(dy, hwin[:, :, :, None], axis=3)[:, :, :, 0]

    flat = hwin * HALF + idx_w.astype(np.int64)    # [core, t, m] in [0, 4096)
    px = (flat % W).astype(np.float32)
    py = (flat // W).astype(np.float32)
    inside = (px > 0) & (px < W - 1) & (py > 0) & (py < H - 1)
    off_x = np.where(inside, np.sign(dx_w).astype(np.float32) * 0.25, 0.0).astype(np.float32)
    off_y = np.where(inside, np.sign(dy_w).astype(np.float32) * 0.25, 0.0).astype(np.float32)
    lx = (px + 0.5 + off_x) * 4.0                  # landmark x
    ly = (py + 0.5 + off_y) * 4.0

    # eyes[b] = [x96, y96, x97, y97];  m = 2*b_local + l
    lx = lx.reshape(N_CORES, 2, BPC, 2)            # [core, t, b_local, l]
    ly = ly.reshape(N_CORES, 2, BPC, 2)
    eyes = np.empty((2, B, 4), np.float32)
    for t in range(2):
        exy = np.stack(
            [lx[:, t, :, 0], ly[:, t, :, 0], lx[:, t, :, 1], ly[:, t, :, 1]],
            axis=-1,
        )                                          # [core, b_local, 4]
        eyes[t] = exy.reshape(B, 4)

    # MSE: col 2, partition r = 8b + 4l + 2h + g  ->  l = (r >> 2) & 1
    ms = pk.astype(np.float64)[:, :, 2].reshape(N_CORES, BPC, 2, 4)
    left = ms[:, :, 0, :].sum() / (B * HW)
    right = ms[:, :, 1, :].sum() / (B * HW)
    eye_loss = np.float32(left + right)

    return eye_loss, eyes[0], eyes[1]
